# revision 11
# baseline (speedup 1.0000x reference)
"""C3D loss kernel for Trainium2 (8 NeuronCores, Bass/Tile).

Sharding: pure data parallel over B*2 = 8 shards (each image split into
top/bottom 176-row halves). Each core computes a partial sum of the loss
numerator; host combines and divides by the valid count.

Layout: partitions = 122 column blocks of 10 pixels (3+3 col halo -> 16
stored cols per block); free dims = (rows, 16). Every spatial shift (the
5x5 window and the normal central differences) is a free-dim offset, which
keeps all engine accesses at partition start 0 (a hardware requirement).

Window phase: channel-stacked tensors (3 channels x ~41 blocks on
partitions, 3 column groups built by SBUF->SBUF DMA) let the per-channel
subtract/square/product run as single wide instructions; channel sums run
on the tensor engine via fp16 embedding matmuls into PSUM, and the 25-
offset per-pixel accumulation runs on the tensor engine too (identity
matmul, PSUM accumulate). Squared differences are scaled by 0.1 inside the
Square activation so they fit fp16 (exp scale compensates exactly).

Out-of-image semantics (must match the reference's zero-pad + `vs` mask):
normals come from zero-padded xyz; the window-phase pred cloud is then
overwritten at out-of-image rows/cols with a poison value (per-core row
strips + column strips, small DMAs) so exp underflows to exactly 0
wherever the reference's `vs` is 0.
"""
import sys

sys.path.insert(0, "/opt/trn_rl_repo")

import numpy as np
from contextlib import ExitStack

import bass_rust
import concourse.bass as bass
import concourse.tile as tile
from concourse import bacc, mybir
from concourse.bass_utils import run_bass_kernel_spmd

F32 = mybir.dt.float32
F16 = mybir.dt.float16
AF = mybir.ActivationFunctionType
ALU = mybir.AluOpType

B, H, W = 4, 352, 1216
R = 2
ELL = 0.05
INV2ELL2 = float(np.float32(1.0 / (2.0 * ELL * ELL)))   # 200.0
EPS = 1e-8
N_CORES = 8

SH = H // 2          # shard rows per core = 176
NT = 2               # row tiles per core
TR = SH // NT        # output rows per tile = 88
HH = TR // 2         # PSUM chunk rows = 44
RB = TR + 6          # stored rows per tile = 94
CB = 10              # cols per block
NB = 122             # blocks
BW = CB + 6          # stored cols per block = 16
SW = CB * (NB - 1) + BW   # slab width = 1226 (slab col j <-> image col j-3)
PZ = 2000.0          # poison depth; (0.1*(PZ-80))**2 ~ 3.7e4 fits fp16
SQS = 0.1            # pre-square scale so fp16 squared diffs stay finite
EXS = float(INV2ELL2 / (SQS * SQS))    # exp scale compensation = 20000
LN14 = float(np.log(0.25))
GRP = [(0, 41), (41, 82), (82, 122)]   # column groups

_prog_cache = {}


def _ap3(base_ap, dims, offset_elems):
    v = base_ap.copy()
    v.ap = bass_rust.VecI64Pair(dims)
    v.offset = v.offset + offset_elems
    return v


def _build_program():
    nc = bacc.Bacc("TRN2", target_bir_lowering=False, debug=False,
                   num_devices=N_CORES)

    for v in (EPS, LN14):
        t = nc.alloc_sbuf_tensor(f"const-f32-{v}", [128, 1], F32)
        nc.gpsimd.memset(t.ap(), v)
        nc.const_aps.aps[(F32, v)] = t.ap()
    nc.all_engine_barrier()

    dp_d = nc.dram_tensor("dp", [SH + 6, SW], F32, kind="ExternalInput").ap()
    dg_d = nc.dram_tensor("dg", [SH + 6, SW], F32, kind="ExternalInput").ap()
    xy1_d = nc.dram_tensor("xy1", [3, SH + 6, SW], F32, kind="ExternalInput").ap()
    mk_d = nc.dram_tensor("mk", [SH, SW], F32, kind="ExternalInput").ap()
    st_d = nc.dram_tensor("strip", [2, 3, NB, 3, BW], F32,
                          kind="ExternalInput").ap()
    pz_d = nc.dram_tensor("pzc", [3, RB, 2], F32, kind="ExternalInput").ap()
    id_d = nc.dram_tensor("idm", [NB, NB], F16, kind="ExternalInput").ap()
    e16_d = nc.dram_tensor("emb16", [3, 123, NB], F16, kind="ExternalInput").ap()
    out_d = nc.dram_tensor("out", [128, NT], F32, kind="ExternalOutput").ap()

    def slab_view(dram_ap, row0, nrows):
        return _ap3(dram_ap, [[CB, NB], [SW, nrows], [1, BW]], row0 * SW)

    with tile.TileContext(nc) as tc, ExitStack() as ctx:
        pool = ctx.enter_context(tc.tile_pool(name="p", bufs=1))
        psum = ctx.enter_context(tc.tile_pool(name="ps", bufs=1, space="PSUM"))
        idt = pool.tile([NB, NB], F16, name="idt")
        nc.sync.dma_start(out=idt[:], in_=id_d[:])
        e16 = pool.tile([123, 3 * NB], F16, name="e16")
        for g in range(3):
            nc.sync.dma_start(out=e16[:, g * NB:(g + 1) * NB], in_=e16_d[g])

        for t in range(NT):
            r0 = t * TR

            # ---------------- input loads ----------------
            dpt = pool.tile([NB, RB, BW], F32, name="dpt")
            nc.sync.dma_start(out=dpt[:], in_=slab_view(dp_d, r0, RB))
            dgt = pool.tile([NB, RB, BW], F32, name="dgt")
            nc.sync.dma_start(out=dgt[:], in_=slab_view(dg_d, r0, RB))
            xy1t = [pool.tile([NB, RB, BW], F32, name=f"xy1t{c}") for c in range(3)]
            for c in range(3):
                nc.sync.dma_start(out=xy1t[c][:], in_=slab_view(xy1_d[c], r0, RB))
            mkt = pool.tile([NB, TR, CB], F32, name="mkt")
            nc.sync.dma_start(
                out=mkt[:],
                in_=_ap3(mk_d, [[CB, NB], [SW, TR], [1, CB]], r0 * SW + 3))

            # ---------------- xyz (zero-padded; feeds normals + stacking) ---
            xp = [pool.tile([NB, RB, BW], F32, name=f"xp{c}") for c in range(3)]
            xg = [pool.tile([NB, RB, BW], F32, name=f"xg{c}") for c in range(3)]
            for c in range(3):
                nc.vector.tensor_mul(xp[c][:], xy1t[c][:], dpt[:])
                nc.vector.tensor_mul(xg[c][:], xy1t[c][:], dgt[:])

            # ------- stacked window tensors (SBUF->SBUF DMA; runs alongside
            # normals since both only read xp/xg) -------
            xpw, xgsc = [], []
            for g, (b0, b1) in enumerate(GRP):
                nb = b1 - b0
                pp = 3 * nb
                xpg = pool.tile([pp, RB, BW], F32, name=f"xpw{g}")
                xgg = pool.tile([pp, TR, CB], F32, name=f"xgs{g}")
                for c in range(3):
                    nc.sync.dma_start(out=xpg[c * nb:(c + 1) * nb],
                                      in_=xp[c][b0:b1])
                    nc.sync.dma_start(out=xgg[c * nb:(c + 1) * nb],
                                      in_=xg[c][b0:b1, 3:3 + TR, 3:3 + CB])
                # poison: per-core row strips at out-of-image rows
                if t == 0:
                    for c in range(3):
                        nc.sync.dma_start(out=xpg[c * nb:(c + 1) * nb, 0:3, :],
                                          in_=st_d[0, c, b0:b1])
                if t == NT - 1:
                    for c in range(3):
                        nc.sync.dma_start(
                            out=xpg[c * nb:(c + 1) * nb, RB - 3:RB, :],
                            in_=st_d[1, c, b0:b1])
                # poison: out-of-image columns (image cols -2,-1 / 1216,1217)
                if g == 0:
                    nc.sync.dma_start(
                        out=_ap3(xpg, [[nb * RB * BW, 3], [BW, RB], [1, 2]], 1),
                        in_=pz_d[:])
                if g == 2:
                    nc.sync.dma_start(
                        out=_ap3(xpg, [[nb * RB * BW, 3], [BW, RB], [1, 2]],
                                 (nb - 1) * RB * BW + 9),
                        in_=pz_d[:])
                xpw.append(xpg)
                xgsc.append(xgg)

            # ---------------- normals ----------------
            def w3(x, dr, dc):
                return x[:, 1 + dr:93 + dr, 1 + dc:15 + dc]

            nrm = {}
            for key, xc in (("p", xp), ("g", xg)):
                eng = nc.vector if key == "p" else nc.gpsimd
                gx = [pool.tile([NB, 92, 14], F32, name=f"gx{c}") for c in range(3)]
                gy = [pool.tile([NB, 92, 14], F32, name=f"gy{c}") for c in range(3)]
                for c in range(3):
                    nc.vector.tensor_sub(gx[c][:], w3(xc[c], 0, 1),
                                         w3(xc[c], 0, -1))
                    nc.vector.tensor_sub(gy[c][:], w3(xc[c], 1, 0),
                                         w3(xc[c], -1, 0))
                cr = [pool.tile([NB, 92, 14], F32, name=f"cr{c}") for c in range(3)]
                tA = pool.tile([NB, 92, 14], F32, name="tA")
                for c in range(3):
                    a, b = (c + 1) % 3, (c + 2) % 3
                    eng.tensor_mul(cr[c][:], gx[a][:], gy[b][:])
                    eng.tensor_mul(tA[:], gx[b][:], gy[a][:])
                    eng.tensor_sub(cr[c][:], cr[c][:], tA[:])
                q = pool.tile([NB, 92, 14], F32, name="q")
                sqt = pool.tile([NB, 92, 14], F32, name="sqt", tag="tA")
                nc.scalar.activation(q[:], cr[0][:], AF.Square)
                nc.scalar.activation(sqt[:], cr[1][:], AF.Square)
                eng.tensor_add(q[:], q[:], sqt[:])
                nc.scalar.activation(sqt[:], cr[2][:], AF.Square)
                eng.tensor_add(q[:], q[:], sqt[:])
                # w = 0.25/(0.25*sqrt(q) + EPS), matching n/(|n|+eps)
                nc.scalar.activation(q[:], q[:], AF.Sqrt, scale=0.0625)
                nc.scalar.activation(q[:], q[:], AF.Ln, bias=EPS)
                nc.scalar.activation(q[:], q[:], AF.Exp, scale=-1.0, bias=LN14)
                nt_ = [pool.tile([NB, RB, BW], F16, name=f"n{key}{c}")
                       for c in range(3)]
                for c in range(3):
                    nc.vector.tensor_mul(w3(nt_[c], 0, 0), cr[c][:], q[:])
                nrm[key] = nt_
            npn, ngn = nrm["p"], nrm["g"]

            # ------- stacked normals (after normals complete) -------
            nps, ngsc = [], []
            for g, (b0, b1) in enumerate(GRP):
                nb = b1 - b0
                pp = 3 * nb
                npg = pool.tile([pp, RB, BW], F16, name=f"nps{g}", tag=f"xp{g}")
                ngg = pool.tile([pp, TR, CB], F16, name=f"ngs{g}", tag=f"xg{g}")
                for c in range(3):
                    nc.sync.dma_start(out=npg[c * nb:(c + 1) * nb],
                                      in_=npn[c][b0:b1])
                    nc.sync.dma_start(out=ngg[c * nb:(c + 1) * nb],
                                      in_=ngn[c][b0:b1, 3:3 + TR, 3:3 + CB])
                nps.append(npg)
                ngsc.append(ngg)

            # ---------------- window phase ----------------
            accP = [psum.tile([NB, HH, CB], F32, name=f"accP{ch}") for ch in range(2)]
            ndP = [psum.tile([NB, HH, CB], F32, name=f"ndP{ch}") for ch in range(2)]

            def shs(x, dy, dx):
                return x[:, 3 + dy:3 + TR + dy, 3 + dx:3 + CB + dx]

            noff = (2 * R + 1) ** 2
            offs = [(dy, dx) for dy in range(-R, R + 1) for dx in range(-R, R + 1)]
            for oi, (dy, dx) in enumerate(offs):
                d2P = [psum.tile([NB, HH, CB], F32, name=f"d2P{ch}",
                                 tag=f"d2P{ch}", bufs=2) for ch in range(2)]
                kgt = pool.tile([NB, TR, CB], F16, name="kgt", tag="kgt")
                stt = pool.tile([NB, TR, CB], F16, name="stt", tag="stt")
                trm = pool.tile([NB, TR, CB], F16, name="trm", tag="trm")
                sbs = [pool.tile([123, TR, CB], F16, name=f"sbf{g}",
                                 tag=f"sbf{g}", bufs=2) for g in range(3)]
                npr = [pool.tile([123, TR, CB], F16, name=f"npr{g}",
                                 tag=f"npr{g}", bufs=2) for g in range(3)]
                for g, (b0, b1) in enumerate(GRP):
                    pp = 3 * (b1 - b0)
                    seng = nc.gpsimd if g == 2 else nc.vector
                    seng.tensor_sub(sbs[g][0:pp], shs(xpw[g], dy, dx),
                                    xgsc[g][:])
                    nc.scalar.activation(sbs[g][0:pp], sbs[g][0:pp], AF.Square,
                                         scale=SQS)
                    nc.vector.tensor_mul(npr[g][0:pp], shs(nps[g], dy, dx),
                                         ngsc[g][:])
                for g in range(3):
                    pp = 3 * (GRP[g][1] - GRP[g][0])
                    for ch in range(2):
                        rs = slice(ch * HH, (ch + 1) * HH)
                        nc.tensor.matmul(d2P[ch][:], e16[0:pp, g * NB:(g + 1) * NB],
                                         sbs[g][0:pp, rs, :],
                                         start=(g == 0), stop=(g == 2))
                    for ch in range(2):
                        rs = slice(ch * HH, (ch + 1) * HH)
                        nc.tensor.matmul(ndP[ch][:], e16[0:pp, g * NB:(g + 1) * NB],
                                         npr[g][0:pp, rs, :],
                                         start=(g == 0), stop=(g == 2))
                for ch in range(2):
                    rs = slice(ch * HH, (ch + 1) * HH)
                    nc.scalar.activation(kgt[:, rs, :], d2P[ch][:], AF.Exp,
                                         scale=-EXS)
                    nc.scalar.activation(stt[:, rs, :], ndP[ch][:], AF.Abs)
                nc.vector.tensor_scalar(stt[:], stt[:], 1.9, 0.1,
                                        ALU.mult, ALU.add)
                nc.vector.tensor_mul(trm[:], stt[:], kgt[:])
                for ch in range(2):
                    rs = slice(ch * HH, (ch + 1) * HH)
                    nc.tensor.matmul(accP[ch][:], idt[:], trm[:, rs, :],
                                     start=(oi == 0), stop=(oi == noff - 1))

            # ---------------- masked reduction ----------------
            pv = pool.tile([NB, TR, CB], F32, name="pv", tag="pv")
            nc.vector.tensor_mul(pv[:, 0:HH, :], accP[0][:], mkt[:, 0:HH, :])
            nc.vector.tensor_mul(pv[:, HH:TR, :], accP[1][:], mkt[:, HH:TR, :])
            red = pool.tile([NB, 1], F32, name="red")
            nc.vector.tensor_reduce(red[:], pv[:], mybir.AxisListType.XY,
                                    ALU.add)
            nc.sync.dma_start(out=out_d[0:NB, t:t + 1], in_=red[:])

    nc.compile()
    return nc


def _consts():
    idm = np.eye(NB, dtype=np.float16)
    e = np.zeros((3, 123, NB), dtype=np.float16)
    for g, (b0, b1) in enumerate(GRP):
        nb = b1 - b0
        for c in range(3):
            for b in range(nb):
                e[g, c * nb + b, b0 + b] = 1.0
    return idm, e


def _strips(xy1_b, dp_b, r0_img):
    """Window-phase xp values for slab rows [0:3) and [179:182)."""
    out = np.zeros((2, 3, NB, 3, BW), dtype=np.float32)
    for side, base in ((0, r0_img - 3), (1, r0_img + SH)):
        vals = np.full((3, 3, SW), PZ, dtype=np.float32)
        for i in range(3):
            y = base + i
            if 0 <= y < H:
                row = np.full((3, SW), PZ, dtype=np.float32)
                row[:, 3:3 + W] = xy1_b[:, y, :] * dp_b[y, :]
                row[:, 1:3] = PZ
                row[:, 3 + W:3 + W + 2] = PZ
                vals[:, i, :] = row
        for p in range(NB):
            out[side, :, p, :, :] = vals[:, :, CB * p:CB * p + BW]
    return out


def kernel(depth_pred, depth_gt, xy1_grid, K, mask):
    if "nc" not in _prog_cache:
        _prog_cache["nc"] = _build_program()
    nc = _prog_cache["nc"]

    dp = np.asarray(depth_pred, dtype=np.float32).reshape(B, H, W)
    dg = np.asarray(depth_gt, dtype=np.float32).reshape(B, H, W)
    xy1 = np.asarray(xy1_grid, dtype=np.float32)
    mk = np.asarray(mask).reshape(B, H, W)

    idm, e16 = _consts()
    pzc = np.full((3, RB, 2), PZ, dtype=np.float32)
    in_maps = []
    for core in range(N_CORES):
        b, half = core // 2, core % 2
        r0 = half * SH
        lo, hi = r0 - 3, r0 + SH + 3
        slo, shi = max(lo, 0), min(hi, H)
        dps = np.zeros((SH + 6, SW), dtype=np.float32)
        dgs = np.zeros((SH + 6, SW), dtype=np.float32)
        xys = np.zeros((3, SH + 6, SW), dtype=np.float32)
        dps[slo - lo:shi - lo, 3:3 + W] = dp[b, slo:shi]
        dgs[slo - lo:shi - lo, 3:3 + W] = dg[b, slo:shi]
        xys[:, slo - lo:shi - lo, 3:3 + W] = xy1[b, :, slo:shi]
        mks = np.zeros((SH, SW), dtype=np.float32)
        mks[:, 3:3 + W] = mk[b, r0:r0 + SH]
        in_maps.append({
            "dp": dps, "dg": dgs, "xy1": xys, "mk": mks,
            "strip": _strips(xy1[b], dp[b], r0),
            "pzc": pzc, "idm": idm, "emb16": e16,
        })

    res = run_bass_kernel_spmd(nc, in_maps, list(range(N_CORES)))
    total = 0.0
    for core in range(N_CORES):
        total += res.results[core]["out"][0:NB, :].astype(np.float64).sum()
    nval = float(mk.sum(dtype=np.float64))
    return np.float32(-total / (nval + EPS))


# revision 13
# speedup vs baseline: 1.3170x; 1.3170x over previous
"""C3D loss kernel for Trainium2 (8 NeuronCores, Bass/Tile).

Sharding: pure data parallel over B*2 = 8 shards (each image split into
top/bottom 176-row halves). Each core computes a partial sum of the loss
numerator; host combines and divides by the valid count.

Layout: partitions = 122 column blocks of 10 pixels (3+3 col halo -> 16
stored cols per block); free dims = (rows, 16). Every spatial shift (the
5x5 window and the normal central differences) is a free-dim offset, which
keeps all engine accesses at partition start 0 (a hardware requirement).

Window phase: channel-stacked tensors (3 channels x ~41 blocks on
partitions, 3 column groups built by SBUF->SBUF DMA) let the per-channel
subtract/square/product run as single wide instructions; channel sums run
on the tensor engine via fp16 embedding matmuls into PSUM, and the 25-
offset per-pixel accumulation runs on the tensor engine too (identity
matmul, PSUM accumulate). Squared differences are scaled by 0.1 inside the
Square activation so they fit fp16 (exp scale compensates exactly).

Out-of-image semantics (must match the reference's zero-pad + `vs` mask):
normals come from zero-padded xyz; the window-phase pred cloud is then
overwritten at out-of-image rows/cols with a poison value (per-core row
strips + column strips, small DMAs) so exp underflows to exactly 0
wherever the reference's `vs` is 0.
"""
import sys

sys.path.insert(0, "/opt/trn_rl_repo")

import numpy as np
from contextlib import ExitStack

import bass_rust
import concourse.bass as bass
import concourse.tile as tile
from concourse import bacc, mybir
from concourse.bass_utils import run_bass_kernel_spmd

F32 = mybir.dt.float32
F16 = mybir.dt.float16
AF = mybir.ActivationFunctionType
ALU = mybir.AluOpType

B, H, W = 4, 352, 1216
R = 2
ELL = 0.05
INV2ELL2 = float(np.float32(1.0 / (2.0 * ELL * ELL)))   # 200.0
EPS = 1e-8
N_CORES = 8

SH = H // 2          # shard rows per core = 176
NT = 2               # row tiles per core
TR = SH // NT        # output rows per tile = 88
HH = TR // 2         # PSUM chunk rows = 44
RB = TR + 6          # stored rows per tile = 94
CB = 10              # cols per block
NB = 122             # blocks
BW = CB + 6          # stored cols per block = 16
SW = CB * (NB - 1) + BW   # slab width = 1226 (slab col j <-> image col j-3)
PZ = 2000.0          # poison depth; (0.1*(PZ-80))**2 ~ 3.7e4 fits fp16
SQS = 0.1            # pre-square scale so fp16 squared diffs stay finite
EXS = float(INV2ELL2 / (SQS * SQS))    # exp scale compensation = 20000
LN14 = float(np.log(0.25))
GRP = [(0, 41), (41, 82), (82, 122)]   # column groups

_prog_cache = {}


def _ap3(base_ap, dims, offset_elems):
    v = base_ap.copy()
    v.ap = bass_rust.VecI64Pair(dims)
    v.offset = v.offset + offset_elems
    return v


def _build_program():
    nc = bacc.Bacc("TRN2", target_bir_lowering=False, debug=False,
                   num_devices=N_CORES)

    for v in (EPS, LN14):
        t = nc.alloc_sbuf_tensor(f"const-f32-{v}", [128, 1], F32)
        nc.gpsimd.memset(t.ap(), v)
        nc.const_aps.aps[(F32, v)] = t.ap()
    nc.all_engine_barrier()

    dp_d = nc.dram_tensor("dp", [SH + 6, SW], F32, kind="ExternalInput").ap()
    dg_d = nc.dram_tensor("dg", [SH + 6, SW], F32, kind="ExternalInput").ap()
    xy1_d = nc.dram_tensor("xy1", [3, SH + 6, SW], F32, kind="ExternalInput").ap()
    mk_d = nc.dram_tensor("mk", [SH, SW], F32, kind="ExternalInput").ap()
    st_d = nc.dram_tensor("strip", [2, 3, NB, 3, BW], F32,
                          kind="ExternalInput").ap()
    pz_d = nc.dram_tensor("pzc", [3, RB, 2], F32, kind="ExternalInput").ap()
    id_d = nc.dram_tensor("idm", [NB, NB], F16, kind="ExternalInput").ap()
    e16_d = nc.dram_tensor("emb16", [3, 123, NB], F16, kind="ExternalInput").ap()
    out_d = nc.dram_tensor("out", [128, NT], F32, kind="ExternalOutput").ap()

    def slab_view(dram_ap, row0, nrows):
        return _ap3(dram_ap, [[CB, NB], [SW, nrows], [1, BW]], row0 * SW)

    with tile.TileContext(nc) as tc, ExitStack() as ctx:
        pool = ctx.enter_context(tc.tile_pool(name="p", bufs=1))
        psum = ctx.enter_context(tc.tile_pool(name="ps", bufs=1, space="PSUM"))
        idt = pool.tile([NB, NB], F16, name="idt")
        nc.sync.dma_start(out=idt[:], in_=id_d[:])
        e16 = pool.tile([123, 3 * NB], F16, name="e16")
        for g in range(3):
            nc.sync.dma_start(out=e16[:, g * NB:(g + 1) * NB], in_=e16_d[g])

        for t in range(NT):
            r0 = t * TR

            # ---------------- input loads ----------------
            dpt = pool.tile([NB, RB, BW], F32, name="dpt")
            nc.sync.dma_start(out=dpt[:], in_=slab_view(dp_d, r0, RB))
            dgt = pool.tile([NB, RB, BW], F32, name="dgt")
            nc.sync.dma_start(out=dgt[:], in_=slab_view(dg_d, r0, RB))
            xy1t = [pool.tile([NB, RB, BW], F32, name=f"xy1t{c}") for c in range(3)]
            for c in range(3):
                nc.sync.dma_start(out=xy1t[c][:], in_=slab_view(xy1_d[c], r0, RB))
            mkt = pool.tile([NB, TR, CB], F32, name="mkt")
            nc.sync.dma_start(
                out=mkt[:],
                in_=_ap3(mk_d, [[CB, NB], [SW, TR], [1, CB]], r0 * SW + 3))

            # ---------------- xyz (zero-padded; feeds normals + stacking) ---
            xp = [pool.tile([NB, RB, BW], F32, name=f"xp{c}") for c in range(3)]
            xg = [pool.tile([NB, RB, BW], F32, name=f"xg{c}") for c in range(3)]
            for c in range(3):
                nc.vector.tensor_mul(xp[c][:], xy1t[c][:], dpt[:])
                nc.vector.tensor_mul(xg[c][:], xy1t[c][:], dgt[:])

            # ------- stacked window tensors (SBUF->SBUF DMA; runs alongside
            # normals since both only read xp/xg) -------
            xpw, xgsc = [], []
            for g, (b0, b1) in enumerate(GRP):
                nb = b1 - b0
                pp = 3 * nb
                xpg = pool.tile([pp, RB, BW], F32, name=f"xpw{g}")
                xgg = pool.tile([pp, TR, CB], F32, name=f"xgs{g}")
                for c in range(3):
                    nc.sync.dma_start(out=xpg[c * nb:(c + 1) * nb],
                                      in_=xp[c][b0:b1])
                    nc.sync.dma_start(out=xgg[c * nb:(c + 1) * nb],
                                      in_=xg[c][b0:b1, 3:3 + TR, 3:3 + CB])
                # poison: per-core row strips at out-of-image rows
                if t == 0:
                    for c in range(3):
                        nc.sync.dma_start(out=xpg[c * nb:(c + 1) * nb, 0:3, :],
                                          in_=st_d[0, c, b0:b1])
                if t == NT - 1:
                    for c in range(3):
                        nc.sync.dma_start(
                            out=xpg[c * nb:(c + 1) * nb, RB - 3:RB, :],
                            in_=st_d[1, c, b0:b1])
                # poison: out-of-image columns (image cols -2,-1 / 1216,1217)
                if g == 0:
                    nc.sync.dma_start(
                        out=_ap3(xpg, [[nb * RB * BW, 3], [BW, RB], [1, 2]], 1),
                        in_=pz_d[:])
                if g == 2:
                    nc.sync.dma_start(
                        out=_ap3(xpg, [[nb * RB * BW, 3], [BW, RB], [1, 2]],
                                 (nb - 1) * RB * BW + 9),
                        in_=pz_d[:])
                xpw.append(xpg)
                xgsc.append(xgg)

            # ---------------- normals ----------------
            def w3(x, dr, dc):
                return x[:, 1 + dr:93 + dr, 1 + dc:15 + dc]

            nrm = {}
            for key, xc in (("p", xp), ("g", xg)):
                eng = nc.vector if key == "p" else nc.gpsimd
                gx = [pool.tile([NB, 92, 14], F32, name=f"gx{c}") for c in range(3)]
                gy = [pool.tile([NB, 92, 14], F32, name=f"gy{c}") for c in range(3)]
                for c in range(3):
                    nc.vector.tensor_sub(gx[c][:], w3(xc[c], 0, 1),
                                         w3(xc[c], 0, -1))
                    nc.vector.tensor_sub(gy[c][:], w3(xc[c], 1, 0),
                                         w3(xc[c], -1, 0))
                cr = [pool.tile([NB, 92, 14], F32, name=f"cr{c}") for c in range(3)]
                tA = pool.tile([NB, 92, 14], F32, name="tA")
                for c in range(3):
                    a, b = (c + 1) % 3, (c + 2) % 3
                    nc.vector.tensor_mul(cr[c][:], gx[a][:], gy[b][:])
                    eng.tensor_mul(tA[:], gx[b][:], gy[a][:])
                    eng.tensor_sub(cr[c][:], cr[c][:], tA[:])
                q = pool.tile([NB, 92, 14], F32, name="q")
                sqt = pool.tile([NB, 92, 14], F32, name="sqt", tag="tA")
                nc.scalar.activation(q[:], cr[0][:], AF.Square)
                nc.scalar.activation(sqt[:], cr[1][:], AF.Square)
                eng.tensor_add(q[:], q[:], sqt[:])
                nc.scalar.activation(sqt[:], cr[2][:], AF.Square)
                eng.tensor_add(q[:], q[:], sqt[:])
                # w = 0.25/(0.25*sqrt(q) + EPS), matching n/(|n|+eps)
                nc.scalar.activation(q[:], q[:], AF.Sqrt, scale=0.0625)
                nc.scalar.activation(q[:], q[:], AF.Ln, bias=EPS)
                nc.scalar.activation(q[:], q[:], AF.Exp, scale=-1.0, bias=LN14)
                nt_ = [pool.tile([NB, RB, BW], F16, name=f"n{key}{c}")
                       for c in range(3)]
                for c in range(3):
                    nc.vector.tensor_mul(w3(nt_[c], 0, 0), cr[c][:], q[:])
                nrm[key] = nt_
            npn, ngn = nrm["p"], nrm["g"]

            # ------- stacked normals (after normals complete) -------
            nps, ngsc = [], []
            for g, (b0, b1) in enumerate(GRP):
                nb = b1 - b0
                pp = 3 * nb
                npg = pool.tile([pp, RB, BW], F16, name=f"nps{g}", tag=f"xp{g}")
                ngg = pool.tile([pp, TR, CB], F16, name=f"ngs{g}", tag=f"xg{g}")
                for c in range(3):
                    nc.sync.dma_start(out=npg[c * nb:(c + 1) * nb],
                                      in_=npn[c][b0:b1])
                    nc.sync.dma_start(out=ngg[c * nb:(c + 1) * nb],
                                      in_=ngn[c][b0:b1, 3:3 + TR, 3:3 + CB])
                nps.append(npg)
                ngsc.append(ngg)

            # ---------------- window phase ----------------
            accP = [psum.tile([NB, HH, CB], F32, name=f"accP{ch}") for ch in range(2)]
            ndP = psum.tile([NB, 2, 512], F32, name="ndP")

            def shs(x, dy, dx):
                return x[:, 3 + dy:3 + TR + dy, 3 + dx:3 + CB + dx]

            noff = (2 * R + 1) ** 2
            offs = [(dy, dx) for dy in range(-R, R + 1) for dx in range(-R, R + 1)]
            for oi, (dy, dx) in enumerate(offs):
                d2P = psum.tile([NB, 2, 512], F32, name="d2P", tag="d2P",
                                bufs=2)
                kgt = pool.tile([NB, TR, CB], F16, name="kgt", tag="kgt")
                stt = pool.tile([NB, TR, CB], F16, name="stt", tag="stt")
                trm = pool.tile([NB, TR, CB], F16, name="trm", tag="trm")
                sbs = [pool.tile([123, TR, CB], F16, name=f"sbf{g}",
                                 tag=f"sbf{g}", bufs=2) for g in range(3)]
                npr = [pool.tile([123, TR, CB], F16, name=f"npr{g}",
                                 tag=f"npr{g}", bufs=2) for g in range(3)]
                for g, (b0, b1) in enumerate(GRP):
                    pp = 3 * (b1 - b0)
                    seng = nc.gpsimd if g == 2 else nc.vector
                    seng.tensor_sub(sbs[g][0:pp], shs(xpw[g], dy, dx),
                                    xgsc[g][:])
                    nc.scalar.activation(sbs[g][0:pp], sbs[g][0:pp], AF.Square,
                                         scale=SQS)
                    nc.vector.tensor_mul(npr[g][0:pp], shs(nps[g], dy, dx),
                                         ngsc[g][:])
                for g in range(3):
                    pp = 3 * (GRP[g][1] - GRP[g][0])
                    for ch in range(2):
                        rs = slice(ch * HH, (ch + 1) * HH)
                        nc.tensor.matmul(d2P[:, ch, 0:HH * CB]
                                         .rearrange("p (r c) -> p r c", c=CB),
                                         e16[0:pp, g * NB:(g + 1) * NB],
                                         sbs[g][0:pp, rs, :],
                                         start=(g == 0), stop=(g == 2))
                    for ch in range(2):
                        rs = slice(ch * HH, (ch + 1) * HH)
                        nc.tensor.matmul(ndP[:, ch, 0:HH * CB]
                                         .rearrange("p (r c) -> p r c", c=CB),
                                         e16[0:pp, g * NB:(g + 1) * NB],
                                         npr[g][0:pp, rs, :],
                                         start=(g == 0), stop=(g == 2))
                nc.scalar.activation(
                    kgt[:].rearrange("p (a r) c -> p a (r c)", a=2),
                    d2P[:, :, 0:HH * CB], AF.Exp, scale=-EXS)
                nc.scalar.activation(
                    stt[:].rearrange("p (a r) c -> p a (r c)", a=2),
                    ndP[:, :, 0:HH * CB], AF.Abs)
                nc.gpsimd.tensor_scalar(stt[:], stt[:], 1.9, 0.1,
                                        ALU.mult, ALU.add)
                nc.vector.tensor_mul(trm[:], stt[:], kgt[:])
                for ch in range(2):
                    rs = slice(ch * HH, (ch + 1) * HH)
                    nc.tensor.matmul(accP[ch][:], idt[:], trm[:, rs, :],
                                     start=(oi == 0), stop=(oi == noff - 1))

            # ---------------- masked reduction ----------------
            pv = pool.tile([NB, TR, CB], F32, name="pv", tag="pv")
            nc.vector.tensor_mul(pv[:, 0:HH, :], accP[0][:], mkt[:, 0:HH, :])
            nc.vector.tensor_mul(pv[:, HH:TR, :], accP[1][:], mkt[:, HH:TR, :])
            red = pool.tile([NB, 1], F32, name="red")
            nc.vector.tensor_reduce(red[:], pv[:], mybir.AxisListType.XY,
                                    ALU.add)
            nc.sync.dma_start(out=out_d[0:NB, t:t + 1], in_=red[:])

    nc.compile()
    return nc


def _consts():
    idm = np.eye(NB, dtype=np.float16)
    e = np.zeros((3, 123, NB), dtype=np.float16)
    for g, (b0, b1) in enumerate(GRP):
        nb = b1 - b0
        for c in range(3):
            for b in range(nb):
                e[g, c * nb + b, b0 + b] = 1.0
    return idm, e


def _strips(xy1_b, dp_b, r0_img):
    """Window-phase xp values for slab rows [0:3) and [179:182)."""
    out = np.zeros((2, 3, NB, 3, BW), dtype=np.float32)
    for side, base in ((0, r0_img - 3), (1, r0_img + SH)):
        vals = np.full((3, 3, SW), PZ, dtype=np.float32)
        for i in range(3):
            y = base + i
            if 0 <= y < H:
                row = np.full((3, SW), PZ, dtype=np.float32)
                row[:, 3:3 + W] = xy1_b[:, y, :] * dp_b[y, :]
                row[:, 1:3] = PZ
                row[:, 3 + W:3 + W + 2] = PZ
                vals[:, i, :] = row
        for p in range(NB):
            out[side, :, p, :, :] = vals[:, :, CB * p:CB * p + BW]
    return out


def kernel(depth_pred, depth_gt, xy1_grid, K, mask):
    if "nc" not in _prog_cache:
        _prog_cache["nc"] = _build_program()
    nc = _prog_cache["nc"]

    dp = np.asarray(depth_pred, dtype=np.float32).reshape(B, H, W)
    dg = np.asarray(depth_gt, dtype=np.float32).reshape(B, H, W)
    xy1 = np.asarray(xy1_grid, dtype=np.float32)
    mk = np.asarray(mask).reshape(B, H, W)

    idm, e16 = _consts()
    pzc = np.full((3, RB, 2), PZ, dtype=np.float32)
    in_maps = []
    for core in range(N_CORES):
        b, half = core // 2, core % 2
        r0 = half * SH
        lo, hi = r0 - 3, r0 + SH + 3
        slo, shi = max(lo, 0), min(hi, H)
        dps = np.zeros((SH + 6, SW), dtype=np.float32)
        dgs = np.zeros((SH + 6, SW), dtype=np.float32)
        xys = np.zeros((3, SH + 6, SW), dtype=np.float32)
        dps[slo - lo:shi - lo, 3:3 + W] = dp[b, slo:shi]
        dgs[slo - lo:shi - lo, 3:3 + W] = dg[b, slo:shi]
        xys[:, slo - lo:shi - lo, 3:3 + W] = xy1[b, :, slo:shi]
        mks = np.zeros((SH, SW), dtype=np.float32)
        mks[:, 3:3 + W] = mk[b, r0:r0 + SH]
        in_maps.append({
            "dp": dps, "dg": dgs, "xy1": xys, "mk": mks,
            "strip": _strips(xy1[b], dp[b], r0),
            "pzc": pzc, "idm": idm, "emb16": e16,
        })

    res = run_bass_kernel_spmd(nc, in_maps, list(range(N_CORES)))
    total = 0.0
    for core in range(N_CORES):
        total += res.results[core]["out"][0:NB, :].astype(np.float64).sum()
    nval = float(mk.sum(dtype=np.float64))
    return np.float32(-total / (nval + EPS))


# revision 22
# speedup vs baseline: 1647.3145x; 1250.7642x over previous
"""C3D loss kernel for Trainium2 (8 NeuronCores, Bass/Tile).

Sharding: pure data parallel over B*2 = 8 shards (each image split into
top/bottom 176-row halves). Each core computes a partial sum of the loss
numerator; host combines and divides by the valid count.

Layout: partitions = 122 column blocks of 10 pixels (3+3 col halo -> 16
stored cols per block); free dims = (rows, 16). Every spatial shift (the
5x5 window and the normal central differences) is a free-dim offset, which
keeps all engine accesses at partition start 0 (a hardware requirement).

Window phase: channel-stacked tensors (3 channels x ~41 blocks on
partitions, 3 column groups built by SBUF->SBUF DMA) let the per-channel
subtract/square/product run as single wide instructions; channel sums run
on the tensor engine via fp16 embedding matmuls into PSUM, and the 25-
offset per-pixel accumulation runs on the tensor engine too (identity
matmul, PSUM accumulate). Squared differences are scaled by 0.1 inside the
Square activation so they fit fp16 (exp scale compensates exactly).

Out-of-image semantics (must match the reference's zero-pad + `vs` mask):
normals come from zero-padded xyz; the window-phase pred cloud is then
overwritten at out-of-image rows/cols with a poison value (per-core row
strips + column strips, small DMAs) so exp underflows to exactly 0
wherever the reference's `vs` is 0.
"""
import sys

sys.path.insert(0, "/opt/trn_rl_repo")

import numpy as np
from contextlib import ExitStack

import bass_rust
import concourse.bass as bass
import concourse.tile as tile
from concourse import bacc, mybir
from concourse.bass_utils import run_bass_kernel_spmd

F32 = mybir.dt.float32
F16 = mybir.dt.float16
AF = mybir.ActivationFunctionType
ALU = mybir.AluOpType

B, H, W = 4, 352, 1216
R = 2
ELL = 0.05
INV2ELL2 = float(np.float32(1.0 / (2.0 * ELL * ELL)))   # 200.0
EPS = 1e-8
N_CORES = 8

SH = H // 2          # shard rows per core = 176
NT = 2               # row tiles per core
TR = SH // NT        # output rows per tile = 88
HH = TR // 2         # PSUM chunk rows = 44
RB = TR + 6          # stored rows per tile = 94
CB = 10              # cols per block
NB = 122             # blocks
BW = CB + 6          # stored cols per block = 16
SW = CB * (NB - 1) + BW   # slab width = 1226 (slab col j <-> image col j-3)
PZ = 2000.0          # poison depth; (0.1*(PZ-80))**2 ~ 3.7e4 fits fp16
SQS = 0.0625         # pre-scale (2^-4, exact) so fp16 sq diffs stay finite
EXS = float(INV2ELL2 / (SQS * SQS))    # exp scale compensation = 20000
LN14 = float(np.log(0.25))
GRP = [(0, 41), (41, 82), (82, 122)]   # column groups

_prog_cache = {}


def _ap3(base_ap, dims, offset_elems):
    v = base_ap.copy()
    v.ap = bass_rust.VecI64Pair(dims)
    v.offset = v.offset + offset_elems
    return v


def _build_program():
    nc = bacc.Bacc("TRN2", target_bir_lowering=False, debug=False,
                   num_devices=N_CORES)

    for v in (EPS, LN14):
        t = nc.alloc_sbuf_tensor(f"const-f32-{v}", [128, 1], F32)
        nc.gpsimd.memset(t.ap(), v)
        nc.const_aps.aps[(F32, v)] = t.ap()
    nc.all_engine_barrier()

    dp_d = nc.dram_tensor("dp", [SH + 6, SW], F32, kind="ExternalInput").ap()
    dg_d = nc.dram_tensor("dg", [SH + 6, SW], F32, kind="ExternalInput").ap()
    xy1_d = nc.dram_tensor("xy1", [3, SH + 6, SW], F32, kind="ExternalInput").ap()
    mk_d = nc.dram_tensor("mk", [SH, SW], F32, kind="ExternalInput").ap()
    st_d = nc.dram_tensor("strip", [2, 3, NB, 3, BW], F32,
                          kind="ExternalInput").ap()
    pz_d = nc.dram_tensor("pzc", [3, RB, 2], F32, kind="ExternalInput").ap()
    id_d = nc.dram_tensor("idm", [NB, NB], F16, kind="ExternalInput").ap()
    e16_d = nc.dram_tensor("emb16", [3, 123, NB], F16, kind="ExternalInput").ap()
    out_d = nc.dram_tensor("out", [128, NT], F32, kind="ExternalOutput").ap()

    def slab_view(dram_ap, row0, nrows):
        return _ap3(dram_ap, [[CB, NB], [SW, nrows], [1, BW]], row0 * SW)

    with tile.TileContext(nc) as tc, ExitStack() as ctx:
        pool = ctx.enter_context(tc.tile_pool(name="p", bufs=1))
        psum = ctx.enter_context(tc.tile_pool(name="ps", bufs=1, space="PSUM"))
        idt = pool.tile([NB, NB], F16, name="idt")
        nc.sync.dma_start(out=idt[:], in_=id_d[:])
        e16 = pool.tile([123, 3 * NB], F16, name="e16")
        for g in range(3):
            nc.sync.dma_start(out=e16[:, g * NB:(g + 1) * NB], in_=e16_d[g])

        for t in range(NT):
            r0 = t * TR

            # ---------------- input loads ----------------
            dpt = pool.tile([NB, RB, BW], F32, name="dpt")
            nc.sync.dma_start(out=dpt[:], in_=slab_view(dp_d, r0, RB))
            dgt = pool.tile([NB, RB, BW], F32, name="dgt")
            nc.sync.dma_start(out=dgt[:], in_=slab_view(dg_d, r0, RB))
            xy1t = [pool.tile([NB, RB, BW], F32, name=f"xy1t{c}") for c in range(3)]
            for c in range(3):
                nc.sync.dma_start(out=xy1t[c][:], in_=slab_view(xy1_d[c], r0, RB))
            mkt = pool.tile([NB, TR, CB], F32, name="mkt")
            nc.sync.dma_start(
                out=mkt[:],
                in_=_ap3(mk_d, [[CB, NB], [SW, TR], [1, CB]], r0 * SW + 3))

            # ---------------- xyz (zero-padded; feeds normals + stacking) ---
            xp = [pool.tile([NB, RB, BW], F32, name=f"xp{c}") for c in range(3)]
            xg = [pool.tile([NB, RB, BW], F32, name=f"xg{c}") for c in range(3)]
            for c in range(3):
                nc.vector.tensor_mul(xp[c][:], xy1t[c][:], dpt[:])
                nc.vector.tensor_mul(xg[c][:], xy1t[c][:], dgt[:])

            # ------- stacked window tensors (SBUF->SBUF DMA; runs alongside
            # normals since both only read xp/xg) -------
            xpw, xgsc = [], []
            for g, (b0, b1) in enumerate(GRP):
                nb = b1 - b0
                pp = 3 * nb
                xpg = pool.tile([pp, 92, 14], F32, name=f"xpw{g}")
                xgg = pool.tile([pp, TR, CB], F32, name=f"xgs{g}")
                for c in range(3):
                    nc.sync.dma_start(out=xpg[c * nb:(c + 1) * nb],
                                      in_=xp[c][b0:b1, 1:93, 1:15])
                    nc.sync.dma_start(out=xgg[c * nb:(c + 1) * nb],
                                      in_=xg[c][b0:b1, 3:3 + TR, 3:3 + CB])
                # poison: per-core row strips at out-of-image rows
                if t == 0:
                    for c in range(3):
                        nc.sync.dma_start(out=xpg[c * nb:(c + 1) * nb, 0:2, :],
                                          in_=st_d[0, c, b0:b1, 1:3, 1:15])
                if t == NT - 1:
                    for c in range(3):
                        nc.sync.dma_start(
                            out=xpg[c * nb:(c + 1) * nb, 90:92, :],
                            in_=st_d[1, c, b0:b1, 0:2, 1:15])
                # poison: out-of-image columns (image cols -2,-1 / 1216,1217)
                if g == 0:
                    nc.sync.dma_start(
                        out=_ap3(xpg, [[nb * 92 * 14, 3], [14, 92], [1, 2]], 0),
                        in_=pz_d[:, 0:92, :])
                if g == 2:
                    nc.sync.dma_start(
                        out=_ap3(xpg, [[nb * 92 * 14, 3], [14, 92], [1, 2]],
                                 (nb - 1) * 92 * 14 + 8),
                        in_=pz_d[:, 0:92, :])
                nc.vector.tensor_scalar_mul(xpg[:], xpg[:], SQS)
                nc.vector.tensor_scalar_mul(xgg[:], xgg[:], SQS)
                xpw.append(xpg)
                xgsc.append(xgg)

            # ---------------- normals ----------------
            def w3(x, dr, dc):
                return x[:, 1 + dr:93 + dr, 1 + dc:15 + dc]

            nrm = {}
            for key, xc in (("p", xp), ("g", xg)):
                eng = nc.vector if key == "p" else nc.gpsimd
                gx = [pool.tile([NB, 92, 14], F32, name=f"gx{c}") for c in range(3)]
                gy = [pool.tile([NB, 92, 14], F32, name=f"gy{c}") for c in range(3)]
                for c in range(3):
                    nc.vector.tensor_sub(gx[c][:], w3(xc[c], 0, 1),
                                         w3(xc[c], 0, -1))
                    nc.vector.tensor_sub(gy[c][:], w3(xc[c], 1, 0),
                                         w3(xc[c], -1, 0))
                cr = [pool.tile([NB, 92, 14], F32, name=f"cr{c}") for c in range(3)]
                tA = pool.tile([NB, 92, 14], F32, name="tA")
                for c in range(3):
                    a, b = (c + 1) % 3, (c + 2) % 3
                    nc.vector.tensor_mul(cr[c][:], gx[a][:], gy[b][:])
                    eng.tensor_mul(tA[:], gx[b][:], gy[a][:])
                    eng.tensor_sub(cr[c][:], cr[c][:], tA[:])
                q = pool.tile([NB, 92, 14], F32, name="q")
                sqt = pool.tile([NB, 92, 14], F32, name="sqt", tag="tA")
                nc.scalar.activation(q[:], cr[0][:], AF.Square)
                nc.scalar.activation(sqt[:], cr[1][:], AF.Square)
                eng.tensor_add(q[:], q[:], sqt[:])
                nc.scalar.activation(sqt[:], cr[2][:], AF.Square)
                eng.tensor_add(q[:], q[:], sqt[:])
                # w = 0.25/(0.25*sqrt(q) + EPS), matching n/(|n|+eps)
                nc.scalar.activation(q[:], q[:], AF.Sqrt, scale=0.0625)
                nc.scalar.activation(q[:], q[:], AF.Ln, bias=EPS)
                nc.scalar.activation(q[:], q[:], AF.Exp, scale=-1.0, bias=LN14)
                nt_ = [pool.tile([NB, 92, 14], F16, name=f"n{key}{c}")
                       for c in range(3)]
                for c in range(3):
                    nc.vector.tensor_mul(nt_[c][:], cr[c][:], q[:])
                nrm[key] = nt_
            npn, ngn = nrm["p"], nrm["g"]

            # ------- stacked normals (after normals complete) -------
            nps, ngsc = [], []
            for g, (b0, b1) in enumerate(GRP):
                nb = b1 - b0
                pp = 3 * nb
                npg = pool.tile([pp, 92, 14], F16, name=f"nps{g}")
                ngg = pool.tile([pp, TR, CB], F16, name=f"ngs{g}")
                for c in range(3):
                    nc.sync.dma_start(out=npg[c * nb:(c + 1) * nb],
                                      in_=npn[c][b0:b1])
                    nc.sync.dma_start(out=ngg[c * nb:(c + 1) * nb],
                                      in_=ngn[c][b0:b1, 2:2 + TR, 2:2 + CB])
                nps.append(npg)
                ngsc.append(ngg)

            # ---------------- window phase ----------------
            accP = [psum.tile([NB, HH, CB], F32, name=f"accP{ch}") for ch in range(2)]
            ndP = psum.tile([NB, 2, 512], F32, name="ndP")

            def shs(x, dy, dx):
                return x[:, 2 + dy:2 + TR + dy, 2 + dx:2 + CB + dx]

            noff = (2 * R + 1) ** 2
            offs = [(dy, dx) for dy in range(-R, R + 1) for dx in range(-R, R + 1)]
            for oi, (dy, dx) in enumerate(offs):
                d2P = psum.tile([NB, 2, 512], F32, name="d2P", tag="d2P",
                                bufs=2)
                kgt = pool.tile([NB, TR, CB], F16, name="kgt", tag="kgt")
                stt = pool.tile([NB, TR, CB], F16, name="stt", tag="stt")
                trm = pool.tile([NB, TR, CB], F16, name="trm", tag="trm")
                sbs = [pool.tile([123, TR, CB], F16, name=f"sbf{g}",
                                 tag=f"sbf{g}", bufs=2) for g in range(3)]
                npr = [pool.tile([123, TR, CB], F16, name=f"npr{g}",
                                 tag=f"npr{g}", bufs=2) for g in range(3)]
                for g, (b0, b1) in enumerate(GRP):
                    pp = 3 * (b1 - b0)
                    seng = nc.gpsimd if g == 2 else nc.vector
                    seng.tensor_sub(sbs[g][0:pp], shs(xpw[g], dy, dx),
                                    xgsc[g][:])
                    if g == 2:
                        nc.vector.tensor_mul(sbs[g][0:pp], sbs[g][0:pp],
                                             sbs[g][0:pp])
                    else:
                        nc.scalar.activation(sbs[g][0:pp], sbs[g][0:pp],
                                             AF.Square)
                    nc.vector.tensor_mul(npr[g][0:pp], shs(nps[g], dy, dx),
                                         ngsc[g][:])
                for g in range(3):
                    pp = 3 * (GRP[g][1] - GRP[g][0])
                    for ch in range(2):
                        rs = slice(ch * HH, (ch + 1) * HH)
                        nc.tensor.matmul(d2P[:, ch, 0:HH * CB]
                                         .rearrange("p (r c) -> p r c", c=CB),
                                         e16[0:pp, g * NB:(g + 1) * NB],
                                         sbs[g][0:pp, rs, :],
                                         start=(g == 0), stop=(g == 2))
                    for ch in range(2):
                        rs = slice(ch * HH, (ch + 1) * HH)
                        nc.tensor.matmul(ndP[:, ch, 0:HH * CB]
                                         .rearrange("p (r c) -> p r c", c=CB),
                                         e16[0:pp, g * NB:(g + 1) * NB],
                                         npr[g][0:pp, rs, :],
                                         start=(g == 0), stop=(g == 2))
                nc.scalar.activation(
                    kgt[:].rearrange("p (a r) c -> p a (r c)", a=2),
                    d2P[:, :, 0:HH * CB], AF.Exp, scale=-EXS)
                nc.scalar.activation(
                    stt[:].rearrange("p (a r) c -> p a (r c)", a=2),
                    ndP[:, :, 0:HH * CB], AF.Abs)
                nc.gpsimd.tensor_scalar(stt[:], stt[:], 1.9, 0.1,
                                        ALU.mult, ALU.add)
                nc.vector.tensor_mul(trm[:], stt[:], kgt[:])
                for ch in range(2):
                    rs = slice(ch * HH, (ch + 1) * HH)
                    nc.tensor.matmul(accP[ch][:], idt[:], trm[:, rs, :],
                                     start=(oi == 0), stop=(oi == noff - 1))

            # ---------------- masked reduction ----------------
            nc.vector.tensor_mul(mkt[:, 0:HH, :], accP[0][:], mkt[:, 0:HH, :])
            nc.vector.tensor_mul(mkt[:, HH:TR, :], accP[1][:], mkt[:, HH:TR, :])
            red = pool.tile([NB, 1], F32, name="red")
            nc.vector.tensor_reduce(red[:], mkt[:], mybir.AxisListType.XY,
                                    ALU.add)
            nc.sync.dma_start(out=out_d[0:NB, t:t + 1], in_=red[:])

    nc.compile()
    return nc


def _consts():
    idm = np.eye(NB, dtype=np.float16)
    e = np.zeros((3, 123, NB), dtype=np.float16)
    for g, (b0, b1) in enumerate(GRP):
        nb = b1 - b0
        for c in range(3):
            for b in range(nb):
                e[g, c * nb + b, b0 + b] = 1.0
    return idm, e


def _strips(xy1_b, dp_b, r0_img):
    """Window-phase xp values for slab rows [0:3) and [179:182)."""
    out = np.zeros((2, 3, NB, 3, BW), dtype=np.float32)
    for side, base in ((0, r0_img - 3), (1, r0_img + SH)):
        vals = np.full((3, 3, SW), PZ, dtype=np.float32)
        for i in range(3):
            y = base + i
            if 0 <= y < H:
                row = np.full((3, SW), PZ, dtype=np.float32)
                row[:, 3:3 + W] = xy1_b[:, y, :] * dp_b[y, :]
                row[:, 1:3] = PZ
                row[:, 3 + W:3 + W + 2] = PZ
                vals[:, i, :] = row
        for p in range(NB):
            out[side, :, p, :, :] = vals[:, :, CB * p:CB * p + BW]
    return out


def kernel(depth_pred, depth_gt, xy1_grid, K, mask):
    if "nc" not in _prog_cache:
        _prog_cache["nc"] = _build_program()
    nc = _prog_cache["nc"]

    dp = np.asarray(depth_pred, dtype=np.float32).reshape(B, H, W)
    dg = np.asarray(depth_gt, dtype=np.float32).reshape(B, H, W)
    xy1 = np.asarray(xy1_grid, dtype=np.float32)
    mk = np.asarray(mask).reshape(B, H, W)

    idm, e16 = _consts()
    pzc = np.full((3, RB, 2), PZ, dtype=np.float32)
    in_maps = []
    for core in range(N_CORES):
        b, half = core // 2, core % 2
        r0 = half * SH
        lo, hi = r0 - 3, r0 + SH + 3
        slo, shi = max(lo, 0), min(hi, H)
        dps = np.zeros((SH + 6, SW), dtype=np.float32)
        dgs = np.zeros((SH + 6, SW), dtype=np.float32)
        xys = np.zeros((3, SH + 6, SW), dtype=np.float32)
        dps[slo - lo:shi - lo, 3:3 + W] = dp[b, slo:shi]
        dgs[slo - lo:shi - lo, 3:3 + W] = dg[b, slo:shi]
        xys[:, slo - lo:shi - lo, 3:3 + W] = xy1[b, :, slo:shi]
        mks = np.zeros((SH, SW), dtype=np.float32)
        mks[:, 3:3 + W] = mk[b, r0:r0 + SH]
        in_maps.append({
            "dp": dps, "dg": dgs, "xy1": xys, "mk": mks,
            "strip": _strips(xy1[b], dp[b], r0),
            "pzc": pzc, "idm": idm, "emb16": e16,
        })

    res = run_bass_kernel_spmd(nc, in_maps, list(range(N_CORES)))
    total = 0.0
    for core in range(N_CORES):
        total += res.results[core]["out"][0:NB, :].astype(np.float64).sum()
    nval = float(mk.sum(dtype=np.float64))
    return np.float32(-total / (nval + EPS))
